# revision 19
# baseline (speedup 1.0000x reference)
"""Trainium2 Bass kernel for nn_DenseAttention_85074712199779.

reference computation (B=8, N=8192, D=512, H=8, DH=64):
    x   = hs * cos + rotate_half(hs) * sin          # RoPE
    q   = x @ W.T                                   # dense projection
    kv  = einsum('bnhd,bnhe->bhde', xh, xh)         # per-head K^T V
    out = einsum('bnhd,bhde->bnhe', qh, kv)         # per-head Q (K^T V)

Sharding: sequence dim N split across the 8 NeuronCores (1024 rows/core,
all batches).  kv needs a cross-core sum -> two small bf16 AllReduces
(512 KB each) that overlap the q-projection matmuls.  All matmuls run in
bf16 (fp32 PSUM accumulation); inputs are pre-cast to bf16 on the host.

Key difference vs the earlier version: x is transposed for the
q-projection with the DMA crossbar (dma_start_transpose, 16x128 xbar
tiles) instead of PE transpose-matmuls.  That removes 256 PE transposes
(~33k PE cycles) and 64 PSUM->SBUF copies per core.  Elementwise work is
spread explicitly: RoPE sin-mults on Pool (gpsimd), cos-mult + add on
DVE (vector), q/out PSUM evacuations on ACT (scalar) + DVE.

Device schedule per core (b = batch):
  - A1(b): DMA hs, RoPE (Pool+DVE), kv = x_hp^T @ x_hp in PSUM,
    evac kv (DVE), DMA kv partial out, xbar-transpose x -> xt (2 DMAs)
  - A2(b): stage1 qT = WT.T @ xt -> resident bf16 qT (evac on ACT)
  - order: A1*3, then A1(b)/A2(b-2) interleaved; AllReduce(kv[0:4])
    after A1(3), AllReduce(kv[4:8]) after A1(7); readbacks on Pool queue
  - B(b): out[m,e] = qT_chunk.T @ blockdiag(kv head-pair), DMA out bf16
    per half batch.
"""

import sys

if "/opt/trn_rl_repo" not in sys.path:
    sys.path.insert(0, "/opt/trn_rl_repo")

import numpy as np
import ml_dtypes

import concourse.bass as bass
import concourse.mybir as mybir
import concourse.tile as tile
from concourse import bacc
from concourse.tile_rust import add_dep_helper

B = 8          # batch
N = 8192       # sequence
D = 512        # hidden
NCORES = 8
R = N // NCORES          # rows per core (1024)
CH = R // 128            # 128-row chunks per batch per core (8)
MB_CH = 4                # chunks per m-block
NMB = CH // MB_CH        # m-blocks per batch (2)
LAG = 2                  # batches stage1 trails stage2
GROUP = 4                # batches per kv AllReduce
BF16 = mybir.dt.bfloat16
F32 = mybir.dt.float32

_CACHE: dict = {}


def _build():
    nc = bacc.Bacc(trn_type="TRN2", num_devices=NCORES)

    hs_ext = nc.declare_dram_parameter("hs", [B, R, D], BF16, isOutput=False)
    cos_ext = nc.declare_dram_parameter("cosb", [R, D], BF16, isOutput=False)
    sin_ext = nc.declare_dram_parameter("sins", [R, D], BF16, isOutput=False)
    wt_ext = nc.declare_dram_parameter("wt", [D, D], BF16, isOutput=False)
    out_ext = nc.declare_dram_parameter("out", [B, R, D], BF16, isOutput=True)

    kv_part = nc.dram_tensor("kv_part", [B, 4, 2, 64, 64], BF16)
    kv_red = nc.dram_tensor("kv_red", [B, 4, 2, 64, 64], BF16, addr_space="Shared")

    rg = [list(range(NCORES))]

    with tile.TileContext(nc) as tc:
        with (
            tc.tile_pool(name="singles", bufs=1) as singles,
            tc.tile_pool(name="hs_p", bufs=4) as hs_p,
            tc.tile_pool(name="x_p", bufs=3) as x_p,
            tc.tile_pool(name="xt_p", bufs=8) as xt_p,
            tc.tile_pool(name="kvs_p", bufs=2) as kvs_p,
            tc.tile_pool(name="out_p", bufs=3) as out_p,
            tc.tile_pool(name="q_ps", bufs=3, space="PSUM") as q_ps,
            tc.tile_pool(name="kv_ps", bufs=2, space="PSUM") as kv_ps,
            tc.tile_pool(name="o_ps", bufs=3, space="PSUM") as o_ps,
        ):
            # ---- resident inputs; first m-block slices go first so RoPE
            # and the first kv matmuls start as early as possible ----
            cos_sb = singles.tile([128, CH, D], BF16, name="cos_sb")
            sin_sb = singles.tile([128, CH, D], BF16, name="sin_sb")
            cos_r = cos_ext.rearrange("(c p) d -> p c d", p=128)
            sin_r = sin_ext.rearrange("(c p) d -> p c d", p=128)

            qT_sb = singles.tile([128, 4, B * R], BF16, name="qT_sb")
            kvblk = singles.tile([128, B, 4, 128], BF16, name="kvblk")
            wt_sb = singles.tile([128, 4, D], BF16, name="wt_sb")

            xt_tiles = {}
            hs_tiles = {}
            kv_writers = []
            tp_instrs = []

            def prefetch_hs(b):
                """hs load triggers for batch b (on the SP queue, BEFORE any
                blocking transpose wait of the current batch)."""
                if b >= B or b in hs_tiles:
                    return
                hs_r = hs_ext[b].rearrange("(c p) d -> p c d", p=128)
                ts = []
                for mb in range(NMB):
                    hs_t = hs_p.tile([128, MB_CH, D], BF16, name="hs_t")
                    nc.sync.dma_start(
                        out=hs_t, in_=hs_r[:, mb * MB_CH:(mb + 1) * MB_CH, :])
                    ts.append(hs_t)
                hs_tiles[b] = ts

            def emit_a1(b):
                """RoPE + kv accumulation + xbar transpose, batch b."""
                x_t = x_p.tile([128, CH, D], BF16, name="x_t")
                kvp = kv_ps.tile([128, 4, 128], F32, name="kvp")
                prefetch_hs(b + 1)
                hs_ts = hs_tiles.pop(b)
                for mb in range(NMB):
                    hs_t = hs_ts[mb]
                    # finer slices for batch 0 so the first kv matmul can
                    # start after only half an m-block of RoPE
                    nsl = 2 if (b == 0 and mb == 0) else 1
                    for s in range(nsl):
                        w = MB_CH // nsl
                        cs = slice(mb * MB_CH + s * w, mb * MB_CH + (s + 1) * w)
                        hsl = slice(s * w, (s + 1) * w)
                        # RoPE: x = hs*cos + swap_half(hs)*sin_signed
                        nc.vector.tensor_tensor(
                            x_t[:, cs, 0:256], hs_t[:, hsl, 256:512],
                            sin_sb[:, cs, 0:256], mybir.AluOpType.mult)
                        nc.vector.tensor_tensor(
                            x_t[:, cs, 256:512], hs_t[:, hsl, 0:256],
                            sin_sb[:, cs, 256:512], mybir.AluOpType.mult)
                        nc.vector.tensor_tensor(
                            hs_t[:, hsl, :], hs_t[:, hsl, :], cos_sb[:, cs, :],
                            mybir.AluOpType.mult)
                        nc.vector.tensor_tensor(
                            x_t[:, cs, :], x_t[:, cs, :], hs_t[:, hsl, :],
                            mybir.AluOpType.add)
                        for cc in range(w):
                            c = cs.start + cc
                            for hp in range(4):
                                xs = x_t[:, c, hp * 128:(hp + 1) * 128]
                                nc.tensor.matmul(
                                    kvp[:, hp, :], xs, xs,
                                    start=(c == 0 and hp == 0),
                                    stop=(c == CH - 1 and hp == 3))
                    # xbar transpose of the m-block: xt[p, c*4+db, j] =
                    # x[(mb*4+c)*128 + j, db*128 + p]
                    xt_t = xt_p.tile([128, MB_CH * 4, 128], BF16, name="xt_t")
                    tp = nc.sync.dma_start_transpose(
                        out=xt_t,
                        in_=x_t[:, mb * MB_CH:(mb + 1) * MB_CH, :])
                    tp_instrs.append(tp)
                    xt_tiles[(b, mb)] = xt_t
                # evacuate kv partial (diagonal 64x64 blocks) on ACT so the
                # DVE RoPE pipeline never waits on PE kv completion
                kv_sb = kvs_p.tile([128, 4, 128], BF16, name="kv_sb")
                nc.scalar.copy(out=kv_sb, in_=kvp)
                # kv_part writes on the ACT queue (right after the evac that
                # produces them) so they never block hs prefetches on SP
                d0 = nc.scalar.dma_start(
                    out=kv_part[b, :, 0].rearrange("h d e -> d h e"),
                    in_=kv_sb[0:64, :, 0:64])
                d1 = nc.scalar.dma_start(
                    out=kv_part[b, :, 1].rearrange("h d e -> d h e"),
                    in_=kv_sb[64:128, :, 64:128])
                kv_writers.extend([d0, d1])

            def emit_a2(b):
                """stage1 qT for batch b from the xbar-transposed x."""
                for mb in range(NMB):
                    xt_t = xt_tiles.pop((b, mb))
                    xt_r = xt_t.rearrange("p (c db) j -> p db c j", db=4)
                    for eb in range(4):
                        qp = q_ps.tile([128, MB_CH * 128], F32, name="qp")
                        for db in range(4):
                            nc.tensor.matmul(
                                qp, wt_sb[:, db, eb * 128:(eb + 1) * 128],
                                xt_r[:, db],
                                start=(db == 0), stop=(db == 3))
                        nc.scalar.copy(
                            out=qT_sb[:, eb, b * R + mb * MB_CH * 128:
                                      b * R + (mb + 1) * MB_CH * 128],
                            in_=qp)

            colls = []

            def emit_allreduce(g0, g1):
                coll = nc.gpsimd.collective_compute(
                    "AllReduce", mybir.AluOpType.add, replica_groups=rg,
                    ins=[kv_part[g0:g1]], outs=[kv_red[g0:g1]])
                for w in kv_writers[2 * g0:2 * g1]:
                    add_dep_helper(coll.ins, w.ins, reason="allreduce after kv dma")
                if not colls:
                    # the framework serializes xbar transposes against
                    # collectives in trigger order; force the first AllReduce
                    # AFTER every transpose so no transpose ends up waiting
                    # for a 15-25us collective to finish
                    for t in tp_instrs:
                        add_dep_helper(coll.ins, t.ins,
                                       reason="allreduce after transposes")
                colls.append((g0, g1, coll))

            def emit_readbacks():
                # on the Pool queue: nothing else needs Pool after RoPE(7),
                # so the wait for the 2nd AllReduce blocks nothing
                for g0, g1, coll in colls:
                    r0 = nc.gpsimd.dma_start(
                        out=kvblk[0:64, g0:g1, :, 0:64],
                        in_=kv_red[g0:g1, :, 0].rearrange("b h d e -> d b h e"))
                    r1 = nc.gpsimd.dma_start(
                        out=kvblk[64:128, g0:g1, :, 64:128],
                        in_=kv_red[g0:g1, :, 1].rearrange("b h d e -> d b h e"))
                    add_dep_helper(r0.ins, coll.ins, reason="rb after allreduce")
                    add_dep_helper(r1.ins, coll.ins, reason="rb after allreduce")

            # ---------------- phase A ----------------
            # The tile framework serializes xbar DMA transposes against
            # collectives, so BOTH AllReduces are emitted after all A1s
            # (= after every transpose).  Phase A throughput is gated by
            # the DVE RoPE (7.3us/batch), so interleaving A2(0..4) fills
            # the PE while A2(5..7) is deferred to cover the AllReduces.
            # A2(b-LAG) is emitted BEFORE A1(b): its inputs are LAG batches
            # old, so the in-order PE queue always has independent matmuls
            # ahead of kv matmuls that may wait on fresh RoPE output.
            # head: first m-block slices of hs(0)/cos/sin go first so RoPE
            # and the first kv matmuls start as early as possible
            hs0_r = hs_ext[0].rearrange("(c p) d -> p c d", p=128)
            hs0a = hs_p.tile([128, MB_CH, D], BF16, name="hs_t")
            hs0b = hs_p.tile([128, MB_CH, D], BF16, name="hs_t")
            hs_tiles[0] = [hs0a, hs0b]
            nc.sync.dma_start(out=hs0a[:, 0:2, :], in_=hs0_r[:, 0:2, :])
            nc.scalar.dma_start(out=sin_sb[:, 0:MB_CH, :], in_=sin_r[:, 0:MB_CH, :])
            nc.sync.dma_start(out=cos_sb[:, 0:MB_CH, :], in_=cos_r[:, 0:MB_CH, :])
            nc.sync.dma_start(out=hs0a[:, 2:4, :], in_=hs0_r[:, 2:4, :])
            nc.sync.dma_start(out=hs0b, in_=hs0_r[:, MB_CH:CH, :])
            nc.scalar.dma_start(out=sin_sb[:, MB_CH:CH, :], in_=sin_r[:, MB_CH:CH, :])
            nc.sync.dma_start(out=cos_sb[:, MB_CH:CH, :], in_=cos_r[:, MB_CH:CH, :])
            # WT strips (host-transposed): wt_sb[:, db, e] = W[e, db*128+p]
            nc.scalar.dma_start(out=wt_sb,
                                in_=wt_ext.rearrange("(b p) e -> p b e", p=128))
            nc.gpsimd.memset(kvblk, 0.0)

            NDEF = 3                      # deferred A2 batches
            for b in range(B):
                if LAG <= b and b - LAG < B - NDEF:
                    emit_a2(b - LAG)
                emit_a1(b)
            emit_allreduce(0, GROUP)
            emit_allreduce(GROUP, B)
            for b in range(B - NDEF, B):
                emit_a2(b)
            emit_readbacks()

            # ---------------- phase B ----------------
            # gpsimd (Pool) has no PSUM access -> DVE/ACT only, DVE-heavy
            evac_eng = {0: nc.vector, 1: nc.scalar, 2: nc.vector,
                        3: nc.vector, 4: nc.scalar, 5: nc.vector,
                        6: nc.vector, 7: nc.scalar}
            for b in range(B):
                out_r = out_ext[b].rearrange("(c p) d -> p c d", p=128)
                out_sb = out_p.tile([128, CH, D], BF16, name="out_sb")
                for c in range(CH):
                    op = o_ps.tile([128, D], F32, name="op")
                    for hp in range(4):
                        nc.tensor.matmul(
                            op[:, hp * 128:(hp + 1) * 128],
                            qT_sb[:, hp, b * R + c * 128:b * R + (c + 1) * 128],
                            kvblk[:, b, hp, :],
                            start=(hp == 0), stop=(hp == 3))
                    eng = evac_eng[c]
                    if eng is nc.scalar:
                        eng.copy(out=out_sb[:, c, :], in_=op)
                    else:
                        eng.tensor_copy(out=out_sb[:, c, :], in_=op)
                    if c == CH // 2 - 1:
                        nc.sync.dma_start(out=out_r[:, 0:CH // 2, :],
                                          in_=out_sb[:, 0:CH // 2, :])
                nc.scalar.dma_start(out=out_r[:, CH // 2:CH, :],
                                    in_=out_sb[:, CH // 2:CH, :])

    nc.compile()
    return nc


def _prep_in_maps(hidden_states, W, cos, sin):
    bf16 = ml_dtypes.bfloat16
    hs = np.ascontiguousarray(hidden_states, dtype=np.float32)
    cos = np.asarray(cos, dtype=np.float32)
    sin = np.asarray(sin, dtype=np.float32)
    sin_signed = np.concatenate([-sin[:, : D // 2], sin[:, D // 2:]], axis=1)
    wt16 = np.ascontiguousarray(np.asarray(W, dtype=np.float32).T).astype(bf16)
    in_maps = []
    for c in range(NCORES):
        rows = slice(c * R, (c + 1) * R)
        in_maps.append({
            "hs": np.ascontiguousarray(hs[:, rows, :]).astype(bf16),
            "cosb": np.ascontiguousarray(cos[rows]).astype(bf16),
            "sins": np.ascontiguousarray(sin_signed[rows]).astype(bf16),
            "wt": wt16,
        })
    return in_maps


def _collect(results):
    out = np.empty((B, N, D), dtype=np.float32)
    for c in range(NCORES):
        out[:, c * R:(c + 1) * R, :] = results[c]["out"].astype(np.float32)
    return out


def kernel(hidden_states, W, cos, sin):
    from concourse.bass_utils import run_bass_kernel_spmd

    nc = _CACHE.get("nc")
    if nc is None:
        nc = _build()
        _CACHE["nc"] = nc

    in_maps = _prep_in_maps(hidden_states, W, cos, sin)
    res = run_bass_kernel_spmd(nc, in_maps, list(range(NCORES)))
    return _collect(res.results)


# revision 21
# speedup vs baseline: 1.0788x; 1.0788x over previous
"""Trainium2 Bass kernel for nn_DenseAttention_85074712199779.

reference computation (B=8, N=8192, D=512, H=8, DH=64):
    x   = hs * cos + rotate_half(hs) * sin          # RoPE
    q   = x @ W.T                                   # dense projection
    kv  = einsum('bnhd,bnhe->bhde', xh, xh)         # per-head K^T V
    out = einsum('bnhd,bhde->bnhe', qh, kv)         # per-head Q (K^T V)

Sharding: sequence dim N split across the 8 NeuronCores (1024 rows/core,
all batches).  kv needs a cross-core sum -> two small bf16 AllReduces
(512 KB each) that overlap the q-projection matmuls.  All matmuls run in
bf16 (fp32 PSUM accumulation); inputs are pre-cast to bf16 on the host.

Key difference vs the earlier version: x is transposed for the
q-projection with the DMA crossbar (dma_start_transpose, 16x128 xbar
tiles) instead of PE transpose-matmuls.  That removes 256 PE transposes
(~33k PE cycles) and 64 PSUM->SBUF copies per core.  Elementwise work is
spread explicitly: RoPE sin-mults on Pool (gpsimd), cos-mult + add on
DVE (vector), q/out PSUM evacuations on ACT (scalar) + DVE.

Device schedule per core (b = batch):
  - A1(b): DMA hs, RoPE (Pool+DVE), kv = x_hp^T @ x_hp in PSUM,
    evac kv (DVE), DMA kv partial out, xbar-transpose x -> xt (2 DMAs)
  - A2(b): stage1 qT = WT.T @ xt -> resident bf16 qT (evac on ACT)
  - order: A1*3, then A1(b)/A2(b-2) interleaved; AllReduce(kv[0:4])
    after A1(3), AllReduce(kv[4:8]) after A1(7); readbacks on Pool queue
  - B(b): out[m,e] = qT_chunk.T @ blockdiag(kv head-pair), DMA out bf16
    per half batch.
"""

import sys

if "/opt/trn_rl_repo" not in sys.path:
    sys.path.insert(0, "/opt/trn_rl_repo")

import numpy as np
import ml_dtypes

import concourse.bass as bass
import concourse.mybir as mybir
import concourse.tile as tile
from concourse import bacc
from concourse.tile_rust import add_dep_helper

B = 8          # batch
N = 8192       # sequence
D = 512        # hidden
NCORES = 8
R = N // NCORES          # rows per core (1024)
CH = R // 128            # 128-row chunks per batch per core (8)
MB_CH = 4                # chunks per m-block
NMB = CH // MB_CH        # m-blocks per batch (2)
LAG = 2                  # batches stage1 trails stage2
GROUP = 4                # batches per kv AllReduce
BF16 = mybir.dt.bfloat16
F32 = mybir.dt.float32

_CACHE: dict = {}


def _build():
    nc = bacc.Bacc(trn_type="TRN2", num_devices=NCORES)

    hs_ext = nc.declare_dram_parameter("hs", [B, R, D], BF16, isOutput=False)
    cos_ext = nc.declare_dram_parameter("cosb", [R, D], BF16, isOutput=False)
    sin_ext = nc.declare_dram_parameter("sins", [R, D], BF16, isOutput=False)
    wt_ext = nc.declare_dram_parameter("wt", [D, D], BF16, isOutput=False)
    out_ext = nc.declare_dram_parameter("out", [B, R, D], BF16, isOutput=True)

    kv_part = nc.dram_tensor("kv_part", [B, 4, 2, 64, 64], BF16)
    kv_red = nc.dram_tensor("kv_red", [B, 4, 2, 64, 64], BF16, addr_space="Shared")

    rg = [list(range(NCORES))]

    with tile.TileContext(nc) as tc:
        with (
            tc.tile_pool(name="singles", bufs=1) as singles,
            tc.tile_pool(name="hs_p", bufs=4) as hs_p,
            tc.tile_pool(name="x_p", bufs=6) as x_p,
            tc.tile_pool(name="xt_p", bufs=8) as xt_p,
            tc.tile_pool(name="kvs_p", bufs=2) as kvs_p,
            tc.tile_pool(name="out_p", bufs=3) as out_p,
            tc.tile_pool(name="q_ps", bufs=3, space="PSUM") as q_ps,
            tc.tile_pool(name="kv_ps", bufs=2, space="PSUM") as kv_ps,
            tc.tile_pool(name="o_ps", bufs=3, space="PSUM") as o_ps,
        ):
            # ---- resident inputs; first m-block slices go first so RoPE
            # and the first kv matmuls start as early as possible ----
            cos_sb = singles.tile([128, CH, D], BF16, name="cos_sb")
            sin_sb = singles.tile([128, CH, D], BF16, name="sin_sb")
            cos_r = cos_ext.rearrange("(c p) d -> p c d", p=128)
            sin_r = sin_ext.rearrange("(c p) d -> p c d", p=128)

            qT_sb = singles.tile([128, 4, B * R], BF16, name="qT_sb")
            kvblk = singles.tile([128, B, 4, 128], BF16, name="kvblk")
            wt_sb = singles.tile([128, 4, D], BF16, name="wt_sb")

            xt_tiles = {}
            hs_tiles = {}
            kv_writers = []
            tp_instrs = []

            def prefetch_hs(b):
                """hs load triggers for batch b (on the SP queue, BEFORE any
                blocking transpose wait of the current batch)."""
                if b >= B or b in hs_tiles:
                    return
                hs_r = hs_ext[b].rearrange("(c p) d -> p c d", p=128)
                ts = []
                for mb in range(NMB):
                    hs_t = hs_p.tile([128, MB_CH, D], BF16, name="hs_t")
                    nc.sync.dma_start(
                        out=hs_t, in_=hs_r[:, mb * MB_CH:(mb + 1) * MB_CH, :])
                    ts.append(hs_t)
                hs_tiles[b] = ts

            def emit_a1(b):
                """RoPE + kv accumulation + xbar transpose, batch b.

                x is a separate tile per m-block: the tile framework tracks
                deps at tile granularity, so a shared x tile would make the
                RoPE write of m-block 1 WAR-wait on the xbar transpose (a
                reader) of m-block 0."""
                kvp = kv_ps.tile([128, 4, 128], F32, name="kvp")
                prefetch_hs(b + 1)
                hs_ts = hs_tiles.pop(b)
                for mb in range(NMB):
                    hs_t = hs_ts[mb]
                    x_t = x_p.tile([128, MB_CH, D], BF16, name="x_t")
                    # finer slices for batch 0 so the first kv matmul can
                    # start after only half an m-block of RoPE
                    nsl = 2 if (b == 0 and mb == 0) else 1
                    for s in range(nsl):
                        w = MB_CH // nsl
                        cs = slice(s * w, (s + 1) * w)
                        gs = slice(mb * MB_CH + s * w, mb * MB_CH + (s + 1) * w)
                        # RoPE: x = hs*cos + swap_half(hs)*sin_signed
                        nc.vector.tensor_tensor(
                            x_t[:, cs, 0:256], hs_t[:, cs, 256:512],
                            sin_sb[:, gs, 0:256], mybir.AluOpType.mult)
                        nc.vector.tensor_tensor(
                            x_t[:, cs, 256:512], hs_t[:, cs, 0:256],
                            sin_sb[:, gs, 256:512], mybir.AluOpType.mult)
                        nc.vector.tensor_tensor(
                            hs_t[:, cs, :], hs_t[:, cs, :], cos_sb[:, gs, :],
                            mybir.AluOpType.mult)
                        nc.vector.tensor_tensor(
                            x_t[:, cs, :], x_t[:, cs, :], hs_t[:, cs, :],
                            mybir.AluOpType.add)
                        for cc in range(w):
                            c = cs.start + cc
                            for hp in range(4):
                                xs = x_t[:, c, hp * 128:(hp + 1) * 128]
                                nc.tensor.matmul(
                                    kvp[:, hp, :], xs, xs,
                                    start=(mb == 0 and c == 0 and hp == 0),
                                    stop=(mb == NMB - 1 and c == MB_CH - 1
                                          and hp == 3))
                    # xbar transpose of the m-block: xt[p, c*4+db, j] =
                    # x[(mb*4+c)*128 + j, db*128 + p]
                    xt_t = xt_p.tile([128, MB_CH * 4, 128], BF16, name="xt_t")
                    tp = nc.sync.dma_start_transpose(out=xt_t, in_=x_t)
                    tp_instrs.append(tp)
                    xt_tiles[(b, mb)] = xt_t
                # evacuate kv partial (diagonal 64x64 blocks) on ACT so the
                # DVE RoPE pipeline never waits on PE kv completion
                kv_sb = kvs_p.tile([128, 4, 128], BF16, name="kv_sb")
                nc.scalar.copy(out=kv_sb, in_=kvp)
                # kv_part writes on the ACT queue (right after the evac that
                # produces them) so they never block hs prefetches on SP
                d0 = nc.scalar.dma_start(
                    out=kv_part[b, :, 0].rearrange("h d e -> d h e"),
                    in_=kv_sb[0:64, :, 0:64])
                d1 = nc.scalar.dma_start(
                    out=kv_part[b, :, 1].rearrange("h d e -> d h e"),
                    in_=kv_sb[64:128, :, 64:128])
                kv_writers.extend([d0, d1])

            def emit_a2(b):
                """stage1 qT for batch b from the xbar-transposed x."""
                for mb in range(NMB):
                    xt_t = xt_tiles.pop((b, mb))
                    xt_r = xt_t.rearrange("p (c db) j -> p db c j", db=4)
                    for eb in range(4):
                        qp = q_ps.tile([128, MB_CH * 128], F32, name="qp")
                        for db in range(4):
                            nc.tensor.matmul(
                                qp, wt_sb[:, db, eb * 128:(eb + 1) * 128],
                                xt_r[:, db],
                                start=(db == 0), stop=(db == 3))
                        nc.scalar.copy(
                            out=qT_sb[:, eb, b * R + mb * MB_CH * 128:
                                      b * R + (mb + 1) * MB_CH * 128],
                            in_=qp)

            colls = []

            def emit_allreduce(g0, g1):
                coll = nc.gpsimd.collective_compute(
                    "AllReduce", mybir.AluOpType.add, replica_groups=rg,
                    ins=[kv_part[g0:g1]], outs=[kv_red[g0:g1]])
                for w in kv_writers[2 * g0:2 * g1]:
                    add_dep_helper(coll.ins, w.ins, reason="allreduce after kv dma")
                if not colls:
                    # the framework serializes xbar transposes against
                    # collectives in trigger order; force the first AllReduce
                    # AFTER every transpose so no transpose ends up waiting
                    # for a 15-25us collective to finish
                    for t in tp_instrs:
                        add_dep_helper(coll.ins, t.ins,
                                       reason="allreduce after transposes")
                colls.append((g0, g1, coll))

            def emit_readbacks():
                # on the Pool queue: nothing else needs Pool after RoPE(7),
                # so the wait for the 2nd AllReduce blocks nothing
                for g0, g1, coll in colls:
                    r0 = nc.gpsimd.dma_start(
                        out=kvblk[0:64, g0:g1, :, 0:64],
                        in_=kv_red[g0:g1, :, 0].rearrange("b h d e -> d b h e"))
                    r1 = nc.gpsimd.dma_start(
                        out=kvblk[64:128, g0:g1, :, 64:128],
                        in_=kv_red[g0:g1, :, 1].rearrange("b h d e -> d b h e"))
                    add_dep_helper(r0.ins, coll.ins, reason="rb after allreduce")
                    add_dep_helper(r1.ins, coll.ins, reason="rb after allreduce")

            # ---------------- phase A ----------------
            # The tile framework serializes xbar DMA transposes against
            # collectives, so BOTH AllReduces are emitted after all A1s
            # (= after every transpose).  Phase A throughput is gated by
            # the DVE RoPE (7.3us/batch), so interleaving A2(0..4) fills
            # the PE while A2(5..7) is deferred to cover the AllReduces.
            # A2(b-LAG) is emitted BEFORE A1(b): its inputs are LAG batches
            # old, so the in-order PE queue always has independent matmuls
            # ahead of kv matmuls that may wait on fresh RoPE output.
            # head: first m-block slices of hs(0)/cos/sin go first so RoPE
            # and the first kv matmuls start as early as possible
            hs0_r = hs_ext[0].rearrange("(c p) d -> p c d", p=128)
            hs0a = hs_p.tile([128, MB_CH, D], BF16, name="hs_t")
            hs0b = hs_p.tile([128, MB_CH, D], BF16, name="hs_t")
            hs_tiles[0] = [hs0a, hs0b]
            nc.sync.dma_start(out=hs0a[:, 0:2, :], in_=hs0_r[:, 0:2, :])
            nc.scalar.dma_start(out=sin_sb[:, 0:MB_CH, :], in_=sin_r[:, 0:MB_CH, :])
            nc.sync.dma_start(out=cos_sb[:, 0:MB_CH, :], in_=cos_r[:, 0:MB_CH, :])
            nc.sync.dma_start(out=hs0a[:, 2:4, :], in_=hs0_r[:, 2:4, :])
            nc.sync.dma_start(out=hs0b, in_=hs0_r[:, MB_CH:CH, :])
            nc.scalar.dma_start(out=sin_sb[:, MB_CH:CH, :], in_=sin_r[:, MB_CH:CH, :])
            nc.sync.dma_start(out=cos_sb[:, MB_CH:CH, :], in_=cos_r[:, MB_CH:CH, :])
            # WT strips (host-transposed): wt_sb[:, db, e] = W[e, db*128+p]
            nc.scalar.dma_start(out=wt_sb,
                                in_=wt_ext.rearrange("(b p) e -> p b e", p=128))
            nc.gpsimd.memset(kvblk, 0.0)

            NDEF = 3                      # deferred A2 batches
            for b in range(B):
                if LAG <= b and b - LAG < B - NDEF:
                    emit_a2(b - LAG)
                emit_a1(b)
            emit_allreduce(0, GROUP)
            emit_allreduce(GROUP, B)
            for b in range(B - NDEF, B):
                emit_a2(b)
            emit_readbacks()

            # ---------------- phase B ----------------
            # gpsimd (Pool) has no PSUM access -> DVE/ACT only, DVE-heavy
            evac_eng = {0: nc.vector, 1: nc.scalar, 2: nc.vector,
                        3: nc.vector, 4: nc.scalar, 5: nc.vector,
                        6: nc.vector, 7: nc.scalar}
            for b in range(B):
                out_r = out_ext[b].rearrange("(c p) d -> p c d", p=128)
                out_sb = out_p.tile([128, CH, D], BF16, name="out_sb")
                for c in range(CH):
                    op = o_ps.tile([128, D], F32, name="op")
                    for hp in range(4):
                        nc.tensor.matmul(
                            op[:, hp * 128:(hp + 1) * 128],
                            qT_sb[:, hp, b * R + c * 128:b * R + (c + 1) * 128],
                            kvblk[:, b, hp, :],
                            start=(hp == 0), stop=(hp == 3))
                    eng = evac_eng[c]
                    if eng is nc.scalar:
                        eng.copy(out=out_sb[:, c, :], in_=op)
                    else:
                        eng.tensor_copy(out=out_sb[:, c, :], in_=op)
                    if c == CH // 2 - 1:
                        nc.sync.dma_start(out=out_r[:, 0:CH // 2, :],
                                          in_=out_sb[:, 0:CH // 2, :])
                nc.scalar.dma_start(out=out_r[:, CH // 2:CH, :],
                                    in_=out_sb[:, CH // 2:CH, :])

    nc.compile()
    return nc


def _prep_in_maps(hidden_states, W, cos, sin):
    bf16 = ml_dtypes.bfloat16
    hs = np.ascontiguousarray(hidden_states, dtype=np.float32)
    cos = np.asarray(cos, dtype=np.float32)
    sin = np.asarray(sin, dtype=np.float32)
    sin_signed = np.concatenate([-sin[:, : D // 2], sin[:, D // 2:]], axis=1)
    wt16 = np.ascontiguousarray(np.asarray(W, dtype=np.float32).T).astype(bf16)
    in_maps = []
    for c in range(NCORES):
        rows = slice(c * R, (c + 1) * R)
        in_maps.append({
            "hs": np.ascontiguousarray(hs[:, rows, :]).astype(bf16),
            "cosb": np.ascontiguousarray(cos[rows]).astype(bf16),
            "sins": np.ascontiguousarray(sin_signed[rows]).astype(bf16),
            "wt": wt16,
        })
    return in_maps


def _collect(results):
    out = np.empty((B, N, D), dtype=np.float32)
    for c in range(NCORES):
        out[:, c * R:(c + 1) * R, :] = results[c]["out"].astype(np.float32)
    return out


def kernel(hidden_states, W, cos, sin):
    from concourse.bass_utils import run_bass_kernel_spmd

    nc = _CACHE.get("nc")
    if nc is None:
        nc = _build()
        _CACHE["nc"] = nc

    in_maps = _prep_in_maps(hidden_states, W, cos, sin)
    res = run_bass_kernel_spmd(nc, in_maps, list(range(NCORES)))
    return _collect(res.results)


# revision 24
# speedup vs baseline: 1.1417x; 1.0583x over previous
"""Trainium2 Bass kernel for nn_DenseAttention_85074712199779.

reference computation (B=8, N=8192, D=512, H=8, DH=64):
    x   = hs * cos + rotate_half(hs) * sin          # RoPE
    q   = x @ W.T                                   # dense projection
    kv  = einsum('bnhd,bnhe->bhde', xh, xh)         # per-head K^T V
    out = einsum('bnhd,bhde->bnhe', qh, kv)         # per-head Q (K^T V)

Sharding: sequence dim N split across the 8 NeuronCores (1024 rows/core,
all batches).  kv needs a cross-core sum -> two small bf16 AllReduces
(512 KB each) that overlap the q-projection matmuls.  All matmuls run in
bf16 (fp32 PSUM accumulation); inputs are pre-cast to bf16 on the host.

Key difference vs the earlier version: x is transposed for the
q-projection with the DMA crossbar (dma_start_transpose, 16x128 xbar
tiles) instead of PE transpose-matmuls.  That removes 256 PE transposes
(~33k PE cycles) and 64 PSUM->SBUF copies per core.  Elementwise work is
spread explicitly: RoPE sin-mults on Pool (gpsimd), cos-mult + add on
DVE (vector), q/out PSUM evacuations on ACT (scalar) + DVE.

Device schedule per core (b = batch):
  - A1(b): DMA hs, RoPE (Pool+DVE), kv = x_hp^T @ x_hp in PSUM,
    evac kv (DVE), DMA kv partial out, xbar-transpose x -> xt (2 DMAs)
  - A2(b): stage1 qT = WT.T @ xt -> resident bf16 qT (evac on ACT)
  - order: A1*3, then A1(b)/A2(b-2) interleaved; AllReduce(kv[0:4])
    after A1(3), AllReduce(kv[4:8]) after A1(7); readbacks on Pool queue
  - B(b): out[m,e] = qT_chunk.T @ blockdiag(kv head-pair), DMA out bf16
    per half batch.
"""

import sys

if "/opt/trn_rl_repo" not in sys.path:
    sys.path.insert(0, "/opt/trn_rl_repo")

import numpy as np
import ml_dtypes

import concourse.bass as bass
import concourse.mybir as mybir
import concourse.tile as tile
from concourse import bacc
from concourse.tile_rust import add_dep_helper

B = 8          # batch
N = 8192       # sequence
D = 512        # hidden
NCORES = 8
R = N // NCORES          # rows per core (1024)
CH = R // 128            # 128-row chunks per batch per core (8)
MB_CH = 4                # chunks per m-block
NMB = CH // MB_CH        # m-blocks per batch (2)
LAG = 2                  # batches stage1 trails stage2
GROUP = 4                # batches per kv AllReduce
BF16 = mybir.dt.bfloat16
F32 = mybir.dt.float32

_CACHE: dict = {}


def _build():
    nc = bacc.Bacc(trn_type="TRN2", num_devices=NCORES)

    hs_ext = nc.declare_dram_parameter("hs", [B, R, D], BF16, isOutput=False)
    cos_ext = nc.declare_dram_parameter("cosb", [R, D], BF16, isOutput=False)
    sin_ext = nc.declare_dram_parameter("sins", [R, D], BF16, isOutput=False)
    wt_ext = nc.declare_dram_parameter("wt", [D, D], BF16, isOutput=False)
    out_ext = nc.declare_dram_parameter("out", [B, R, D], BF16, isOutput=True)

    kv_part = nc.dram_tensor("kv_part", [B, 4, 2, 64, 64], BF16)
    kv_red = nc.dram_tensor("kv_red", [B, 4, 2, 64, 64], BF16, addr_space="Shared")

    rg = [list(range(NCORES))]

    with tile.TileContext(nc) as tc:
        with (
            tc.tile_pool(name="singles", bufs=1) as singles,
            tc.tile_pool(name="hs_p", bufs=6) as hs_p,
            tc.tile_pool(name="x_p", bufs=6) as x_p,
            tc.tile_pool(name="xt_p", bufs=8) as xt_p,
            tc.tile_pool(name="kvs_p", bufs=2) as kvs_p,
            tc.tile_pool(name="out_p", bufs=3) as out_p,
            tc.tile_pool(name="q_ps", bufs=3, space="PSUM") as q_ps,
            tc.tile_pool(name="kv_ps", bufs=2, space="PSUM") as kv_ps,
            tc.tile_pool(name="o_ps", bufs=3, space="PSUM") as o_ps,
        ):
            # ---- resident inputs; first m-block slices go first so RoPE
            # and the first kv matmuls start as early as possible ----
            cos_sb = singles.tile([128, CH, D], BF16, name="cos_sb")
            sin_sb = singles.tile([128, CH, D], BF16, name="sin_sb")
            cos_r = cos_ext.rearrange("(c p) d -> p c d", p=128)
            sin_r = sin_ext.rearrange("(c p) d -> p c d", p=128)

            qT_sb = singles.tile([128, 4, B * R], BF16, name="qT_sb")
            kvblk = singles.tile([128, B, 4, 128], BF16, name="kvblk")
            wt_sb = singles.tile([128, 4, D], BF16, name="wt_sb")

            xt_tiles = {}
            hs_tiles = {}
            kv_writers = []
            tp_instrs = []

            def prefetch_hs(b):
                """hs load triggers for batch b (on the SP queue, BEFORE any
                blocking transpose wait of the current batch)."""
                if b >= B or b in hs_tiles:
                    return
                hs_r = hs_ext[b].rearrange("(c p) d -> p c d", p=128)
                ts = []
                for mb in range(NMB):
                    hs_t = hs_p.tile([128, MB_CH, D], BF16, name="hs_t")
                    nc.sync.dma_start(
                        out=hs_t, in_=hs_r[:, mb * MB_CH:(mb + 1) * MB_CH, :])
                    ts.append(hs_t)
                hs_tiles[b] = ts

            def emit_a1(b):
                """RoPE + kv accumulation + xbar transpose, batch b.

                x is a separate tile per m-block: the tile framework tracks
                deps at tile granularity, so a shared x tile would make the
                RoPE write of m-block 1 WAR-wait on the xbar transpose (a
                reader) of m-block 0."""
                kvp = kv_ps.tile([128, 4, 128], F32, name="kvp")
                prefetch_hs(b + 2)
                hs_ts = hs_tiles.pop(b)
                for mb in range(NMB):
                    hs_t = hs_ts[mb]
                    x_t = x_p.tile([128, MB_CH, D], BF16, name="x_t")
                    # finer slices for batch 0 so the first kv matmul can
                    # start after only half an m-block of RoPE
                    nsl = 2 if (b == 0 and mb == 0) else 1
                    for s in range(nsl):
                        w = MB_CH // nsl
                        cs = slice(s * w, (s + 1) * w)
                        gs = slice(mb * MB_CH + s * w, mb * MB_CH + (s + 1) * w)
                        # RoPE: x = hs*cos + swap_half(hs)*sin_signed
                        nc.vector.tensor_tensor(
                            x_t[:, cs, 0:256], hs_t[:, cs, 256:512],
                            sin_sb[:, gs, 0:256], mybir.AluOpType.mult)
                        nc.vector.tensor_tensor(
                            x_t[:, cs, 256:512], hs_t[:, cs, 0:256],
                            sin_sb[:, gs, 256:512], mybir.AluOpType.mult)
                        nc.vector.tensor_tensor(
                            hs_t[:, cs, :], hs_t[:, cs, :], cos_sb[:, gs, :],
                            mybir.AluOpType.mult)
                        nc.vector.tensor_tensor(
                            x_t[:, cs, :], x_t[:, cs, :], hs_t[:, cs, :],
                            mybir.AluOpType.add)
                        for cc in range(w):
                            c = cs.start + cc
                            for hp in range(4):
                                xs = x_t[:, c, hp * 128:(hp + 1) * 128]
                                nc.tensor.matmul(
                                    kvp[:, hp, :], xs, xs,
                                    start=(mb == 0 and c == 0 and hp == 0),
                                    stop=(mb == NMB - 1 and c == MB_CH - 1
                                          and hp == 3))
                    # xbar transpose of the m-block: xt[p, c*4+db, j] =
                    # x[(mb*4+c)*128 + j, db*128 + p]
                    xt_t = xt_p.tile([128, MB_CH * 4, 128], BF16, name="xt_t")
                    tp = nc.sync.dma_start_transpose(out=xt_t, in_=x_t)
                    tp_instrs.append(tp)
                    xt_tiles[(b, mb)] = xt_t
                # evacuate kv partial (diagonal 64x64 blocks) on ACT so the
                # DVE RoPE pipeline never waits on PE kv completion
                kv_sb = kvs_p.tile([128, 4, 128], BF16, name="kv_sb")
                nc.scalar.copy(out=kv_sb, in_=kvp)
                # kv_part writes on the ACT queue (right after the evac that
                # produces them) so they never block hs prefetches on SP
                d0 = nc.scalar.dma_start(
                    out=kv_part[b, :, 0].rearrange("h d e -> d h e"),
                    in_=kv_sb[0:64, :, 0:64])
                d1 = nc.scalar.dma_start(
                    out=kv_part[b, :, 1].rearrange("h d e -> d h e"),
                    in_=kv_sb[64:128, :, 64:128])
                kv_writers.extend([d0, d1])

            def emit_a2(b):
                """stage1 qT for batch b from the xbar-transposed x."""
                for mb in range(NMB):
                    xt_t = xt_tiles.pop((b, mb))
                    xt_r = xt_t.rearrange("p (c db) j -> p db c j", db=4)
                    for eb in range(4):
                        qp = q_ps.tile([128, MB_CH * 128], F32, name="qp")
                        for db in range(4):
                            nc.tensor.matmul(
                                qp, wt_sb[:, db, eb * 128:(eb + 1) * 128],
                                xt_r[:, db],
                                start=(db == 0), stop=(db == 3))
                        nc.scalar.copy(
                            out=qT_sb[:, eb, b * R + mb * MB_CH * 128:
                                      b * R + (mb + 1) * MB_CH * 128],
                            in_=qp)

            colls = []

            def emit_allreduce(g0, g1):
                coll = nc.gpsimd.collective_compute(
                    "AllReduce", mybir.AluOpType.add, replica_groups=rg,
                    ins=[kv_part[g0:g1]], outs=[kv_red[g0:g1]])
                for w in kv_writers[2 * g0:2 * g1]:
                    add_dep_helper(coll.ins, w.ins, reason="allreduce after kv dma")
                if not colls:
                    # the framework serializes xbar transposes against
                    # collectives in trigger order; force the first AllReduce
                    # AFTER every transpose so no transpose ends up waiting
                    # for a 15-25us collective to finish
                    for t in tp_instrs:
                        add_dep_helper(coll.ins, t.ins,
                                       reason="allreduce after transposes")
                colls.append((g0, g1, coll))

            def emit_readbacks():
                # on the Pool queue: nothing else needs Pool after RoPE(7),
                # so the wait for the 2nd AllReduce blocks nothing
                for g0, g1, coll in colls:
                    r0 = nc.gpsimd.dma_start(
                        out=kvblk[0:64, g0:g1, :, 0:64],
                        in_=kv_red[g0:g1, :, 0].rearrange("b h d e -> d b h e"))
                    r1 = nc.gpsimd.dma_start(
                        out=kvblk[64:128, g0:g1, :, 64:128],
                        in_=kv_red[g0:g1, :, 1].rearrange("b h d e -> d b h e"))
                    add_dep_helper(r0.ins, coll.ins, reason="rb after allreduce")
                    add_dep_helper(r1.ins, coll.ins, reason="rb after allreduce")

            # ---------------- phase A ----------------
            # The tile framework serializes xbar DMA transposes against
            # collectives, so BOTH AllReduces are emitted after all A1s
            # (= after every transpose).  Phase A throughput is gated by
            # the DVE RoPE (7.3us/batch), so interleaving A2(0..4) fills
            # the PE while A2(5..7) is deferred to cover the AllReduces.
            # A2(b-LAG) is emitted BEFORE A1(b): its inputs are LAG batches
            # old, so the in-order PE queue always has independent matmuls
            # ahead of kv matmuls that may wait on fresh RoPE output.
            # head: first m-block slices of hs(0)/cos/sin go first so RoPE
            # and the first kv matmuls start as early as possible
            hs0_r = hs_ext[0].rearrange("(c p) d -> p c d", p=128)
            hs0a = hs_p.tile([128, MB_CH, D], BF16, name="hs_t")
            hs0b = hs_p.tile([128, MB_CH, D], BF16, name="hs_t")
            hs_tiles[0] = [hs0a, hs0b]
            nc.sync.dma_start(out=hs0a[:, 0:2, :], in_=hs0_r[:, 0:2, :])
            nc.scalar.dma_start(out=sin_sb[:, 0:MB_CH, :], in_=sin_r[:, 0:MB_CH, :])
            nc.sync.dma_start(out=cos_sb[:, 0:MB_CH, :], in_=cos_r[:, 0:MB_CH, :])
            nc.sync.dma_start(out=hs0a[:, 2:4, :], in_=hs0_r[:, 2:4, :])
            nc.sync.dma_start(out=hs0b, in_=hs0_r[:, MB_CH:CH, :])
            nc.scalar.dma_start(out=sin_sb[:, MB_CH:CH, :], in_=sin_r[:, MB_CH:CH, :])
            nc.sync.dma_start(out=cos_sb[:, MB_CH:CH, :], in_=cos_r[:, MB_CH:CH, :])
            # WT strips (host-transposed): wt_sb[:, db, e] = W[e, db*128+p]
            nc.scalar.dma_start(out=wt_sb,
                                in_=wt_ext.rearrange("(b p) e -> p b e", p=128))
            nc.gpsimd.memset(kvblk, 0.0)
            prefetch_hs(1)

            NDEF = 3                      # deferred A2 batches
            for b in range(B):
                if LAG <= b and b - LAG < B - NDEF:
                    emit_a2(b - LAG)
                emit_a1(b)
            emit_allreduce(0, GROUP)
            emit_allreduce(GROUP, B)
            for b in range(B - NDEF, B):
                emit_a2(b)
            emit_readbacks()

            # ---------------- phase B ----------------
            # gpsimd (Pool) has no PSUM access -> DVE/ACT only, DVE-heavy
            evac_eng = {0: nc.vector, 1: nc.scalar, 2: nc.vector,
                        3: nc.vector, 4: nc.scalar, 5: nc.vector,
                        6: nc.vector, 7: nc.scalar}
            for b in range(B):
                out_r = out_ext[b].rearrange("(c p) d -> p c d", p=128)
                out_sb = out_p.tile([128, CH, D], BF16, name="out_sb")
                for c in range(CH):
                    op = o_ps.tile([128, D], F32, name="op")
                    for hp in range(4):
                        nc.tensor.matmul(
                            op[:, hp * 128:(hp + 1) * 128],
                            qT_sb[:, hp, b * R + c * 128:b * R + (c + 1) * 128],
                            kvblk[:, b, hp, :],
                            start=(hp == 0), stop=(hp == 3))
                    eng = evac_eng[c]
                    if eng is nc.scalar:
                        eng.copy(out=out_sb[:, c, :], in_=op)
                    else:
                        eng.tensor_copy(out=out_sb[:, c, :], in_=op)
                    if c == CH // 2 - 1:
                        nc.sync.dma_start(out=out_r[:, 0:CH // 2, :],
                                          in_=out_sb[:, 0:CH // 2, :])
                nc.scalar.dma_start(out=out_r[:, CH // 2:CH, :],
                                    in_=out_sb[:, CH // 2:CH, :])

    nc.compile()
    return nc


def _prep_in_maps(hidden_states, W, cos, sin):
    bf16 = ml_dtypes.bfloat16
    hs = np.ascontiguousarray(hidden_states, dtype=np.float32)
    cos = np.asarray(cos, dtype=np.float32)
    sin = np.asarray(sin, dtype=np.float32)
    sin_signed = np.concatenate([-sin[:, : D // 2], sin[:, D // 2:]], axis=1)
    wt16 = np.ascontiguousarray(np.asarray(W, dtype=np.float32).T).astype(bf16)
    in_maps = []
    for c in range(NCORES):
        rows = slice(c * R, (c + 1) * R)
        in_maps.append({
            "hs": np.ascontiguousarray(hs[:, rows, :]).astype(bf16),
            "cosb": np.ascontiguousarray(cos[rows]).astype(bf16),
            "sins": np.ascontiguousarray(sin_signed[rows]).astype(bf16),
            "wt": wt16,
        })
    return in_maps


def _collect(results):
    out = np.empty((B, N, D), dtype=np.float32)
    for c in range(NCORES):
        out[:, c * R:(c + 1) * R, :] = results[c]["out"].astype(np.float32)
    return out


def kernel(hidden_states, W, cos, sin):
    from concourse.bass_utils import run_bass_kernel_spmd

    nc = _CACHE.get("nc")
    if nc is None:
        nc = _build()
        _CACHE["nc"] = nc

    in_maps = _prep_in_maps(hidden_states, W, cos, sin)
    res = run_bass_kernel_spmd(nc, in_maps, list(range(NCORES)))
    return _collect(res.results)
